# revision 15
# baseline (speedup 1.0000x reference)
"""Self-contained Trainium2 Bass kernel for nn_Classifier_79929341379065.

kernel(**inputs) takes FULL unsharded inputs (as produced by
reference.setup_inputs()) and returns the FULL [B, 1] float32 output.
Internally: pure data parallel over 8 NeuronCores (batch dim of x),
weights replicated.

Architecture notes (hardcoded shapes):
  B=8192, L=16, H=8, DK=DV=32, DM=256, BN=128, V=50000.
  Per core: 1024 batches = 16384 tokens, processed as 128 subtiles of
  128 tokens (each subtile = 8 attention groups of L=16).

Device-side dataflow per subtile (token-major = tokens on partitions):
  gather n  = LN(node_emb)[x]        (table pre-normalized on host, bf16)
  gather em = (node_emb * npm)[x]    (table with row 0 zeroed, bf16)
  nT,emT = PE transpose; qkT = W @ nT (feature-major); v = nT.T @ WvT
  S^T_h  = kh^T.T @ qh^T   (per head, block-diag mask applied after exp)
  P~     = exp(S^T) * mask ;  [ctx~|den] = P~.T @ [v|1] ; ctx = ctx~/den
  dyn path: fc1 -> tanh mlp -> residual -> LN ;  sta path: tanh mlp -> LN
  probs  = sigmoid(((dyn-sta)^2) @ Wcls.T + bcls)
  out    = group-sum(probs*npm)/group-sum(npm) via tiny matmul with
           group-indicator, accumulated in SBUF, divided once at the end.
"""

import os
import sys
import types

import numpy as np

# ---------------------------------------------------------------- constants
B, L = 8192, 16
H, DK, DV = 8, 32, 32
DM, BN, V = 256, 128, 50000
NCORES = 8
P = 128
BC = B // NCORES                  # batches per core (1024)
TOKC = BC * L                     # tokens per core (16384)
NSUB_FULL = TOKC // P             # subtiles per core (128)
GRP = P // L                      # groups per subtile (8)
SCL = 1.0 / np.sqrt(float(DK))
EPS = 1e-5


def _install_ntff_hook():
    """Register the axon NTFF profiling hook if the image's antenv lacks it,
    so run_bass_kernel_spmd(trace=True) works in this container."""
    try:
        import antenv.axon_hooks  # noqa: F401
        return
    except ImportError:
        pass
    try:
        from trn_agent_boot.trn_boot import _ntff_profile_via_ctypes
        hook = _ntff_profile_via_ctypes("/opt/axon/libaxon_pjrt.so")
    except Exception:
        hook = None
    m = types.ModuleType("antenv.axon_hooks")
    m.get_axon_ntff_profile_hook = lambda: hook
    m.set_axon_ntff_profile_hook = lambda h: None
    sys.modules["antenv.axon_hooks"] = m


def _bf16(a):
    import ml_dtypes
    return np.ascontiguousarray(a.astype(ml_dtypes.bfloat16))


def _triv(g, b):
    return bool(np.allclose(g, 1.0, atol=1e-12) and np.allclose(b, 0.0, atol=1e-12))


# ------------------------------------------------------------- host weights
def _prep_consts(w):
    """Fold LN affines into projection weights; build device const arrays."""
    c = {}
    f32 = np.float32

    wq_eff = (w["Wq"] * w["ln1_g"][None, :]) * SCL          # [256,128]
    wk_eff = w["Wk"] * w["ln2_g"][None, :]
    wv_eff = w["Wv"] * w["ln3_g"][None, :]
    cq = (w["ln1_b"] @ w["Wq"].T) * SCL                      # [256]
    ck = w["ln2_b"] @ w["Wk"].T
    cv = w["ln3_b"] @ w["Wv"].T

    c["wqk"] = _bf16(np.concatenate([wq_eff.T, wk_eff.T], axis=1))  # [128,512]

    wv_aug = np.zeros((BN, H * (DV + 1)), f32)               # [128,264]
    for h in range(H):
        wv_aug[:, h * 33:h * 33 + 32] = wv_eff.T[:, h * 32:(h + 1) * 32]
    c["wv"] = _bf16(wv_aug)

    c["wfc1"] = _bf16(w["Wfc1"].T)                           # [HDV=256, DM=256]
    c["p1w1"] = _bf16(w["p1_w1"].T)                          # [DM, DM] lhsT
    c["p1w2"] = _bf16(w["p1_w2"].T)                          # [DM, DM] rhs
    c["p2w1"] = _bf16(w["p2_w1"].T)                          # [BN, DM] lhsT
    c["p2w2"] = _bf16(w["p2_w2"].T)                          # [DM, DM] rhs

    # pre-normalized embedding table (row-wise LN, affine folded into W above)
    tab = np.asarray(w["node_emb"], f32)
    m = tab.mean(axis=1, keepdims=True)
    v = ((tab - m) ** 2).mean(axis=1, keepdims=True)
    c["tabn"] = _bf16((tab - m) / np.sqrt(v + EPS))
    tabe = tab.copy()
    tabe[0, :] = 0.0                                         # x==0 -> npm=0 -> emb*npm=0
    c["tabe"] = _bf16(tabe)

    # block-diag(16) minus eye multiplicative mask, tiled 4x along free dim
    blk = np.zeros((P, P), f32)
    for g in range(GRP):
        blk[g * L:(g + 1) * L, g * L:(g + 1) * L] = 1.0
    blk -= np.eye(P, dtype=f32)
    blk = np.maximum(blk, 0.0)
    c["maskt"] = _bf16(np.tile(blk, (1, 4)))                 # [128,512]

    gind = np.zeros((P, GRP), f32)
    for g in range(GRP):
        gind[g * L:(g + 1) * L, g] = 1.0
    c["gind"] = gind

    hm4 = np.zeros((P, 512), f32)   # head mask: 1 where partition f in head hh
    for hh in range(4):
        hm4[hh * 32:(hh + 1) * 32, hh * P:(hh + 1) * P] = 1.0
    c["hm4"] = _bf16(hm4)

    c["wclsb"] = np.ascontiguousarray(
        np.broadcast_to(np.asarray(w["Wcls"], f32).reshape(1, DM), (P, DM)))
    c["ident"] = _bf16(np.eye(P, dtype=f32))

    flags = {
        "qkb": not (np.allclose(cq, 0.0) and np.allclose(ck, 0.0)),
        "cv": not np.allclose(cv, 0.0),
        "p1b1": not np.allclose(w["p1_b1"], 0.0),
        "p2b1": not np.allclose(w["p2_b1"], 0.0),
        "p1b2": not np.allclose(w["p1_b2"], 0.0),
        "p2b2": not np.allclose(w["p2_b2"], 0.0),
        "p1aff": not _triv(w["p1_lng"], w["p1_lnb"]),
        "c1aff": not _triv(w["lnc1_g"], w["lnc1_b"]),
        "c2aff": not _triv(w["lnc2_g"], w["lnc2_b"]),
    }
    flags["lnc1"] = flags["p1aff"]  # full LN_c1 only needed if p1 affine nontrivial

    if flags["qkb"]:
        cqk = np.zeros((P, 4), f32)
        cqk[:, 0], cqk[:, 1] = cq[0:128], cq[128:256]
        cqk[:, 2], cqk[:, 3] = ck[0:128], ck[128:256]
        c["cqk"] = cqk
    if flags["cv"]:
        cvb = np.zeros((P, H * 33), f32)
        for h in range(H):
            cvb[:, h * 33:h * 33 + 32] = np.broadcast_to(
                cv[h * 32:(h + 1) * 32][None, :], (P, 32))
        c["cvb"] = cvb
    if flags["p1b1"]:
        c["p1b1"] = np.stack([w["p1_b1"][0:128], w["p1_b1"][128:256]], 1).astype(f32)
    if flags["p2b1"]:
        c["p2b1"] = np.stack([w["p2_b1"][0:128], w["p2_b1"][128:256]], 1).astype(f32)
    if flags["p1b2"]:
        c["p1b2b"] = np.broadcast_to(w["p1_b2"][None, :], (P, DM)).astype(f32).copy()
    if flags["p2b2"]:
        c["p2b2b"] = np.broadcast_to(w["p2_b2"][None, :], (P, DM)).astype(f32).copy()
    for nm, fl in (("p1", "p1aff"), ("c1", "c1aff"), ("c2", "c2aff")):
        if flags[fl]:
            gk = {"p1": "p1_lng", "c1": "lnc1_g", "c2": "lnc2_g"}[nm]
            bk = {"p1": "p1_lnb", "c1": "lnc1_b", "c2": "lnc2_b"}[nm]
            c[nm + "gb"] = np.broadcast_to(w[gk][None, :], (P, DM)).astype(f32).copy()
            c[nm + "bb"] = np.broadcast_to(w[bk][None, :], (P, DM)).astype(f32).copy()

    c["_bcls"] = float(np.asarray(w["bcls"]).reshape(-1)[0])
    c["_flags"] = flags
    return c


# ------------------------------------------------------------ device program
def build_nc(flags, bcls, n_sub, stage=8):
    import concourse.bacc as bacc
    import concourse.tile as tile
    import concourse.mybir as mybir
    from concourse import bass

    dt = mybir.dt
    AF = mybir.ActivationFunctionType
    OP = mybir.AluOpType
    IOA = bass.IndirectOffsetOnAxis

    nc = bacc.Bacc()

    # ---- dram tensors
    idxc = nc.dram_tensor("idxc", [P, n_sub], dt.int32, kind="ExternalInput")
    npmc = nc.dram_tensor("npmc", [P, n_sub], dt.float32, kind="ExternalInput")
    tabn = nc.dram_tensor("tabn", [V, BN], dt.bfloat16, kind="ExternalInput")
    tabe = nc.dram_tensor("tabe", [V, BN], dt.bfloat16, kind="ExternalInput")
    wqk_d = nc.dram_tensor("wqk", [BN, 512], dt.bfloat16, kind="ExternalInput")
    wv_d = nc.dram_tensor("wv", [BN, 264], dt.bfloat16, kind="ExternalInput")
    wfc1_d = nc.dram_tensor("wfc1", [DM, DM], dt.bfloat16, kind="ExternalInput")
    p1w1_d = nc.dram_tensor("p1w1", [DM, DM], dt.bfloat16, kind="ExternalInput")
    p1w2_d = nc.dram_tensor("p1w2", [DM, DM], dt.bfloat16, kind="ExternalInput")
    p2w1_d = nc.dram_tensor("p2w1", [BN, DM], dt.bfloat16, kind="ExternalInput")
    p2w2_d = nc.dram_tensor("p2w2", [DM, DM], dt.bfloat16, kind="ExternalInput")
    mask_d = nc.dram_tensor("maskt", [P, 512], dt.bfloat16, kind="ExternalInput")
    hm4_d = nc.dram_tensor("hm4", [P, 512], dt.bfloat16, kind="ExternalInput")
    gind_d = nc.dram_tensor("gind", [P, GRP], dt.float32, kind="ExternalInput")
    wcls_d = nc.dram_tensor("wclsb", [P, DM], dt.float32, kind="ExternalInput")
    ident_d = nc.dram_tensor("ident", [P, P], dt.bfloat16, kind="ExternalInput")
    opt_d = {}
    for nm, shp, cond in [
        ("cqk", [P, 4], flags["qkb"]), ("cvb", [P, 264], flags["cv"]),
        ("p1b1", [P, 2], flags["p1b1"]), ("p2b1", [P, 2], flags["p2b1"]),
        ("p1b2b", [P, DM], flags["p1b2"]), ("p2b2b", [P, DM], flags["p2b2"]),
        ("p1gb", [P, DM], flags["p1aff"]), ("p1bb", [P, DM], flags["p1aff"]),
        ("c1gb", [P, DM], flags["c1aff"]), ("c1bb", [P, DM], flags["c1aff"]),
        ("c2gb", [P, DM], flags["c2aff"]), ("c2bb", [P, DM], flags["c2aff"]),
    ]:
        if cond:
            opt_d[nm] = nc.dram_tensor(nm, shp, dt.float32, kind="ExternalInput")
    outp = nc.dram_tensor("outp", [GRP, n_sub], dt.float32, kind="ExternalOutput")

    with tile.TileContext(nc) as tc:
        import contextlib
        with contextlib.ExitStack() as ctx:
            singles = ctx.enter_context(tc.tile_pool(name="singles", bufs=1))
            io = ctx.enter_context(tc.tile_pool(name="io", bufs=4))
            work = ctx.enter_context(tc.tile_pool(name="work", bufs=3))
            # PSUM budget: 8 banks total -> big:2 + med:3 + tr:3
            ps_s = ctx.enter_context(tc.tile_pool(name="ps_s", bufs=2, space="PSUM"))
            ps_m = ctx.enter_context(tc.tile_pool(name="ps_m", bufs=3, space="PSUM"))
            ps_t = ctx.enter_context(tc.tile_pool(name="ps_t", bufs=2, space="PSUM"))

            # ---- load constants
            def load(d, shape, dtp):
                t = singles.tile(shape, dtp, name=d.name + "_sb")
                nc.sync.dma_start(t[:], d[:, :])
                return t

            idx_sb = load(idxc, [P, n_sub], dt.int32)
            npm_sb = load(npmc, [P, n_sub], dt.float32)
            wqk = load(wqk_d, [BN, 512], dt.bfloat16)
            wv = load(wv_d, [BN, 264], dt.bfloat16)
            mask_sb = load(mask_d, [P, 512], dt.bfloat16)
            hm4_sb = load(hm4_d, [P, 512], dt.bfloat16)
            gind_sb = load(gind_d, [P, GRP], dt.float32)
            wcls_sb = load(wcls_d, [P, DM], dt.float32)
            ident = load(ident_d, [P, P], dt.bfloat16)
            wfc1 = [None, None]
            p1w1 = [None, None]
            p1w2 = [None, None]
            p2w2 = [None, None]
            for k in range(2):
                wfc1[k] = singles.tile([P, DM], dt.bfloat16, name=f"wfc1_{k}")
                nc.sync.dma_start(wfc1[k][:], wfc1_d[k * P:(k + 1) * P, :])
                p1w1[k] = singles.tile([P, DM], dt.bfloat16, name=f"p1w1_{k}")
                nc.sync.dma_start(p1w1[k][:], p1w1_d[k * P:(k + 1) * P, :])
                p1w2[k] = singles.tile([P, DM], dt.bfloat16, name=f"p1w2_{k}")
                nc.sync.dma_start(p1w2[k][:], p1w2_d[k * P:(k + 1) * P, :])
                p2w2[k] = singles.tile([P, DM], dt.bfloat16, name=f"p2w2_{k}")
                nc.sync.dma_start(p2w2[k][:], p2w2_d[k * P:(k + 1) * P, :])
            p2w1 = load(p2w1_d, [BN, DM], dt.bfloat16)
            osb = {nm: load(d, d.shape, dt.float32) for nm, d in opt_d.items()}

            epst = singles.tile([P, 1], dt.float32, name="epst")
            nc.vector.memset(epst[:], EPS)
            res = singles.tile([GRP, 2 * n_sub], dt.float32, name="res")
            if stage < 8:
                nc.vector.memset(res[:], 1.0)

            # ---- helper: layernorm stats -> (mean_ap, rstd tile)
            def ln_stats(x_ap, tag):
                st6 = work.tile([P, 6], dt.float32, tag=f"st6{tag}", name=f"st6{tag}")
                nc.vector.bn_stats(st6[:], x_ap)
                mv = work.tile([P, 2], dt.float32, tag=f"mv{tag}", name=f"mv{tag}")
                nc.vector.bn_aggr(mv[:], st6[:])
                std = work.tile([P, 1], dt.float32, tag=f"std{tag}", name=f"std{tag}")
                nc.scalar.activation(std[:], mv[:, 1:2], AF.Sqrt, bias=epst[:, 0:1])
                rstd = work.tile([P, 1], dt.float32, tag=f"rs{tag}", name=f"rs{tag}")
                nc.vector.reciprocal(rstd[:], std[:])
                return mv, rstd

            for t in range(n_sub):
                # ---- gathers (pre-normalized and masked-raw embeddings)
                n_bf = io.tile([P, BN], dt.bfloat16, tag="n_bf", name="n_bf")
                nc.gpsimd.indirect_dma_start(
                    out=n_bf[:], out_offset=None, in_=tabn[:, :],
                    in_offset=IOA(ap=idx_sb[:, t:t + 1], axis=0))
                em_bf = io.tile([P, BN], dt.bfloat16, tag="em_bf", name="em_bf")
                nc.gpsimd.indirect_dma_start(
                    out=em_bf[:], out_offset=None, in_=tabe[:, :],
                    in_offset=IOA(ap=idx_sb[:, t:t + 1], axis=0))

                npm = npm_sb[:, t:t + 1]  # [128,1] f32 mask, host-precomputed

                # ---- transposes of n and em (packed into one PSUM tile)
                ne_ps = ps_t.tile([P, 2 * P], dt.bfloat16, tag="tr", name="ne_ps")
                nc.tensor.transpose(ne_ps[:, 0:P], n_bf[:], ident[:])
                nc.tensor.transpose(ne_ps[:, P:2 * P], em_bf[:], ident[:])
                neT = work.tile([P, 2 * P], dt.bfloat16, tag="neT", name="neT")
                nc.vector.tensor_copy(neT[:], ne_ps[:])
                nT, emT = neT[:, 0:P], neT[:, P:2 * P]
                if stage <= 1:
                    continue

                # ---- q/k feature-major [feat, tok]; v token-major (augmented)
                qk_ps = ps_s.tile([P, 512], dt.float32, tag="big", name="qk_ps")
                for j in range(4):
                    nc.tensor.matmul(qk_ps[:, j * P:(j + 1) * P],
                                     lhsT=wqk[:, j * P:(j + 1) * P], rhs=nT)
                qkT = work.tile([P, 512], dt.bfloat16, tag="qkT", name="qkT")
                if flags["qkb"]:
                    for j in range(4):
                        nc.scalar.activation(
                            qkT[:, j * P:(j + 1) * P], qk_ps[:, j * P:(j + 1) * P],
                            AF.Identity, bias=osb["cqk"][:, j:j + 1])
                else:
                    nc.scalar.activation(qkT[:], qk_ps[:], AF.Copy)

                v_ps = ps_m.tile([P, 264], dt.float32, tag="med", name="v_ps")
                nc.tensor.matmul(v_ps[:], lhsT=nT, rhs=wv[:])
                v_aug = work.tile([P, 264], dt.bfloat16, tag="v_aug", name="v_aug")
                nc.scalar.activation(v_aug[:], v_ps[:], AF.Copy)
                va3 = v_aug[:].rearrange("p (h c) -> p h c", c=33)
                if flags["cv"]:
                    nc.vector.tensor_add(v_aug[:], v_aug[:], osb["cvb"][:])
                nc.gpsimd.memset(va3[:, :, 32:33], 1.0)
                if stage <= 2:
                    continue

                # ---- attention scores S^T + exp + mask (2 blocks of 4 heads)
                # S^T for 4 heads in ONE K=128 matmul: lhsT = kT block (full
                # 128 features), rhs = head-masked 4x-replicated q block, so
                # out[j, hh*128+t] = sum_{f in head hh} kT[f,j] qT[f,t].
                PTm = [None, None]
                for b2 in range(2):
                    qh = qkT[:, 128 * b2:128 * (b2 + 1)]
                    q_rep = bass.AP(tensor=qh.tensor, offset=qh.offset,
                                    ap=[qh.ap[0], [0, 4], qh.ap[1]])
                    qT4 = work.tile([P, 512], dt.bfloat16, tag="qT4", name="qT4")
                    nc.vector.tensor_tensor(
                        out=qT4[:].rearrange("p (h t) -> p h t", h=4),
                        in0=q_rep, in1=hm4_sb[:].rearrange("p (h t) -> p h t", h=4),
                        op=OP.mult)
                    s_ps = ps_s.tile([P, 512], dt.float32, tag="big", name="s_ps")
                    nc.tensor.matmul(
                        s_ps[:], lhsT=qkT[:, 256 + 128 * b2:256 + 128 * (b2 + 1)],
                        rhs=qT4[:])
                    pt = work.tile([P, 512], dt.bfloat16, tag="pt", name="pt")
                    nc.scalar.activation(pt[:], s_ps[:], AF.Exp)
                    ptm = work.tile([P, 512], dt.bfloat16, tag="ptm", name="ptm")
                    nc.gpsimd.tensor_tensor(out=ptm[:], in0=pt[:], in1=mask_sb[:],
                                            op=OP.mult)
                    PTm[b2] = ptm
                if stage <= 3:
                    continue

                # ---- PV: [ctx~|den] per head, then normalize
                ca_ps = ps_m.tile([P, 264], dt.float32, tag="med", name="ca_ps")
                for h in range(H):
                    b2, hh = divmod(h, 4)
                    nc.tensor.matmul(
                        ca_ps[:, h * 33:(h + 1) * 33],
                        lhsT=PTm[b2][:, hh * P:(hh + 1) * P],
                        rhs=v_aug[:, h * 33:(h + 1) * 33])
                ca3 = ca_ps[:].rearrange("p (h c) -> p h c", c=33)
                rec = work.tile([P, H], dt.float32, tag="rec", name="rec")
                rec3 = rec[:].rearrange("p (h o) -> p h o", o=1)
                nc.vector.reciprocal(rec3[:], ca3[:, :, 32:33])
                ctx_bf = work.tile([P, 256], dt.bfloat16, tag="ctx", name="ctx_bf")
                cb3 = ctx_bf[:].rearrange("p (h c) -> p h c", c=32)
                nc.vector.tensor_tensor(out=cb3[:], in0=ca3[:, :, 0:32],
                                        in1=rec3.to_broadcast([P, H, 32]),
                                        op=OP.mult)

                # ---- ctx transpose + dyn_in (fc1)
                ct_ps = ps_t.tile([P, 2 * P], dt.bfloat16, tag="tr", name="ct_ps")
                nc.tensor.transpose(ct_ps[:, 0:P], ctx_bf[:, 0:P], ident[:])
                nc.tensor.transpose(ct_ps[:, P:2 * P], ctx_bf[:, P:2 * P], ident[:])
                ctxT = work.tile([P, 2 * P], dt.bfloat16, tag="ctxT", name="ctxT")
                nc.scalar.activation(ctxT[:], ct_ps[:], AF.Copy)
                if stage <= 4:
                    continue

                di_ps = ps_m.tile([P, DM], dt.float32, tag="med", name="di_ps")
                nc.tensor.matmul(di_ps[:], lhsT=ctxT[:, 0:P], rhs=wfc1[0][:],
                                 start=True, stop=False)
                nc.tensor.matmul(di_ps[:], lhsT=ctxT[:, P:2 * P], rhs=wfc1[1][:],
                                 start=False, stop=True)
                di_f = work.tile([P, DM], dt.float32, tag="di_f", name="di_f")
                nc.vector.tensor_scalar_mul(di_f[:], di_ps[:], npm)
                di_bf = work.tile([P, DM], dt.bfloat16, tag="di_bf", name="di_bf")
                nc.scalar.activation(di_bf[:], di_ps[:], AF.Identity,
                                     scale=npm)

                # ---- h1 = tanh(dyn_in @ p1_w1.T + b1)  (feature-major)
                dt_ps = ps_t.tile([P, 2 * P], dt.bfloat16, tag="tr", name="dt_ps")
                nc.tensor.transpose(dt_ps[:, 0:P], di_bf[:, 0:P], ident[:])
                nc.tensor.transpose(dt_ps[:, P:2 * P], di_bf[:, P:2 * P], ident[:])
                dinT = work.tile([P, 2 * P], dt.bfloat16, tag="dinT", name="dinT")
                nc.vector.tensor_copy(dinT[:], dt_ps[:])

                h1_ps = ps_t.tile([P, DM], dt.float32, tag="h1", name="h1_ps", bufs=1)
                for f in range(2):
                    for k in range(2):
                        nc.tensor.matmul(
                            h1_ps[:, f * P:(f + 1) * P],
                            lhsT=p1w1[k][:, f * P:(f + 1) * P],
                            rhs=dinT[:, k * P:(k + 1) * P],
                            start=(k == 0), stop=(k == 1))
                h1T = work.tile([P, DM], dt.bfloat16, tag="h1T", name="h1T")
                if flags["p1b1"]:
                    for f in range(2):
                        nc.scalar.activation(h1T[:, f * P:(f + 1) * P],
                                             h1_ps[:, f * P:(f + 1) * P], AF.Tanh,
                                             bias=osb["p1b1"][:, f:f + 1])
                else:
                    nc.scalar.activation(h1T[:], h1_ps[:], AF.Tanh)

                # ---- u = h1 @ p1_w2.T (+ b2) + dyn_in ; dynamic = LN_p1(u)*npm
                u_ps = ps_m.tile([P, DM], dt.float32, tag="med", name="u_ps")
                nc.tensor.matmul(u_ps[:], lhsT=h1T[:, 0:P], rhs=p1w2[0][:],
                                 start=True, stop=False)
                nc.tensor.matmul(u_ps[:], lhsT=h1T[:, P:2 * P], rhs=p1w2[1][:],
                                 start=False, stop=True)
                u_f = work.tile([P, DM], dt.float32, tag="u_f", name="u_f")
                nc.vector.tensor_add(u_f[:], u_ps[:], di_f[:])
                if flags["p1b2"]:
                    nc.vector.tensor_add(u_f[:], u_f[:], osb["p1b2b"][:])

                mv2, rstd2 = ln_stats(u_f[:], "2")
                dyn_f = work.tile([P, DM], dt.float32, tag="dyn", name="dyn_f")
                if not flags["p1aff"] and not flags["c1aff"]:
                    rsn = work.tile([P, 1], dt.float32, tag="rsn", name="rsn")
                    nc.vector.tensor_mul(rsn[:], rstd2[:], npm)
                    nc.gpsimd.tensor_scalar(
                        out=dyn_f[:], in0=u_f[:], scalar1=mv2[:, 0:1],
                        scalar2=rsn[:, 0:1], op0=OP.subtract, op1=OP.mult)
                else:
                    d0 = work.tile([P, DM], dt.float32, tag="dyn0", name="d0")
                    nc.gpsimd.tensor_scalar(
                        out=d0[:], in0=u_f[:], scalar1=mv2[:, 0:1],
                        scalar2=rstd2[:, 0:1], op0=OP.subtract, op1=OP.mult)
                    if flags["p1aff"]:
                        nc.vector.tensor_mul(d0[:], d0[:], osb["p1gb"][:])
                        nc.vector.tensor_add(d0[:], d0[:], osb["p1bb"][:])
                    nc.vector.tensor_scalar_mul(d0[:], d0[:], npm)
                    if flags["lnc1"]:
                        mvc, rstdc = ln_stats(d0[:], "c1")
                        nc.gpsimd.tensor_scalar(
                            out=dyn_f[:], in0=d0[:], scalar1=mvc[:, 0:1],
                            scalar2=rstdc[:, 0:1], op0=OP.subtract, op1=OP.mult)
                    else:
                        nc.vector.tensor_copy(dyn_f[:], d0[:])
                    if flags["c1aff"]:
                        nc.vector.tensor_mul(dyn_f[:], dyn_f[:], osb["c1gb"][:])
                        nc.vector.tensor_add(dyn_f[:], dyn_f[:], osb["c1bb"][:])

                if stage <= 5:
                    continue

                # ---- static path: h2 = tanh(em @ p2_w1.T + b1) feature-major
                h2_ps = ps_t.tile([P, DM], dt.float32, tag="h1", name="h2_ps", bufs=1)
                for f in range(2):
                    nc.tensor.matmul(h2_ps[:, f * P:(f + 1) * P],
                                     lhsT=p2w1[:, f * P:(f + 1) * P], rhs=emT)
                h2T = work.tile([P, DM], dt.bfloat16, tag="h2T", name="h2T")
                if flags["p2b1"]:
                    for f in range(2):
                        nc.scalar.activation(h2T[:, f * P:(f + 1) * P],
                                             h2_ps[:, f * P:(f + 1) * P], AF.Tanh,
                                             bias=osb["p2b1"][:, f:f + 1])
                else:
                    nc.scalar.activation(h2T[:], h2_ps[:], AF.Tanh)

                st_ps = ps_m.tile([P, DM], dt.float32, tag="med", name="st_ps")
                nc.tensor.matmul(st_ps[:], lhsT=h2T[:, 0:P], rhs=p2w2[0][:],
                                 start=True, stop=False)
                nc.tensor.matmul(st_ps[:], lhsT=h2T[:, P:2 * P], rhs=p2w2[1][:],
                                 start=False, stop=True)
                stm = work.tile([P, DM], dt.float32, tag="stm", name="stm")
                if flags["p2b2"]:
                    nc.vector.tensor_add(stm[:], st_ps[:], osb["p2b2b"][:])
                    nc.vector.tensor_scalar_mul(stm[:], stm[:], npm)
                else:
                    nc.vector.tensor_scalar_mul(stm[:], st_ps[:], npm)

                mv3, rstd3 = ln_stats(stm[:], "3")
                sta_f = work.tile([P, DM], dt.float32, tag="sta", name="sta_f")
                nc.gpsimd.tensor_scalar(
                    out=sta_f[:], in0=stm[:], scalar1=mv3[:, 0:1],
                    scalar2=rstd3[:, 0:1], op0=OP.subtract, op1=OP.mult)
                if flags["c2aff"]:
                    nc.vector.tensor_mul(sta_f[:], sta_f[:], osb["c2gb"][:])
                    nc.vector.tensor_add(sta_f[:], sta_f[:], osb["c2bb"][:])
                if stage <= 6:
                    continue

                # ---- head: sigmoid(sum((dyn-sta)^2 * wcls) + bcls)
                dd = work.tile([P, DM], dt.float32, tag="dd", name="dd")
                nc.gpsimd.tensor_tensor(out=dd[:], in0=dyn_f[:], in1=sta_f[:],
                                        op=OP.subtract)
                dw = work.tile([P, DM], dt.float32, tag="dw", name="dw")
                nc.gpsimd.tensor_tensor(out=dw[:], in0=dd[:], in1=wcls_sb[:],
                                        op=OP.mult)
                wsq = work.tile([P, DM], dt.float32, tag="wsq", name="wsq")
                nc.vector.tensor_mul(wsq[:], dd[:], dw[:])
                ttro = work.tile([P, DM], dt.float32, tag="ttro", name="ttro")
                logit = work.tile([P, 1], dt.float32, tag="logit", name="logit")
                nc.scalar.activation(ttro[:], wsq[:], AF.Copy, accum_out=logit[:])
                probs = work.tile([P, 1], dt.float32, tag="probs", name="probs")
                nc.scalar.activation(probs[:], logit[:], AF.Sigmoid, bias=bcls)
                if stage <= 7:
                    continue
                pn2 = work.tile([P, 2], dt.float32, tag="pn2", name="pn2")
                nc.vector.tensor_mul(pn2[:, 0:1], probs[:], npm)
                nc.gpsimd.tensor_copy(pn2[:, 1:2], npm)

                agg_ps = ps_m.tile([GRP, 2], dt.float32, tag="med", name="agg_ps")
                nc.tensor.matmul(agg_ps[:], lhsT=gind_sb[:], rhs=pn2[:])
                nc.scalar.activation(res[0:GRP, 2 * t:2 * t + 2], agg_ps[:], AF.Copy)

            # ---- final divide + store
            r3 = res[:].rearrange("p (t k) -> p t k", k=2)
            rn = work.tile([GRP, n_sub], dt.float32, tag="rn", name="rn")
            rn3 = rn[:].rearrange("p (t o) -> p t o", o=1)
            nc.vector.reciprocal(rn3[:], r3[:, :, 1:2])
            orow = work.tile([GRP, n_sub], dt.float32, tag="orow", name="orow")
            orow3 = orow[:].rearrange("p (t o) -> p t o", o=1)
            nc.vector.tensor_tensor(out=orow3[:], in0=r3[:, :, 0:1], in1=rn3[:],
                                    op=OP.mult)
            nc.sync.dma_start(outp[:, :], orow[:])

    nc.finalize()
    return nc


# ----------------------------------------------------------------- entry
_NC_CACHE = {}


def kernel(**inputs):
    _install_ntff_hook()
    from concourse.bass_utils import run_bass_kernel_spmd

    n_sub = int(os.environ.get("KBENCH_NSUB", NSUB_FULL))
    consts = _prep_consts(inputs)
    flags = consts.pop("_flags")
    bcls = consts.pop("_bcls")

    stage = int(os.environ.get("KBENCH_STAGE", "8"))
    key = (n_sub, stage, tuple(sorted(flags.items())))
    if key not in _NC_CACHE:
        _NC_CACHE[key] = build_nc(flags, bcls, n_sub, stage)
    nc = _NC_CACHE[key]

    x = np.asarray(inputs["x"]).astype(np.int32)
    in_maps = []
    for c in range(NCORES):
        xc = x[c * BC:(c + 1) * BC].reshape(-1)          # [16384]
        idxc = np.ascontiguousarray(
            xc[:n_sub * P].reshape(n_sub, P).T)          # [128, n_sub]
        m = {"idxc": idxc, "npmc": (idxc != 0).astype(np.float32)}
        m.update(consts)
        in_maps.append(m)

    trace = bool(int(os.environ.get("KBENCH_TRACE", "0")))
    res = run_bass_kernel_spmd(nc, in_maps, core_ids=list(range(NCORES)),
                               trace=trace)
    kernel._last_results = res

    out = np.zeros((B, 1), np.float32)
    for c in range(NCORES):
        oc = res.results[c]["outp"]                      # [8, n_sub]
        out[c * BC:c * BC + n_sub * GRP, 0] = oc.T.reshape(-1)
    return out
